# revision 1
# baseline (speedup 1.0000x reference)
"""AsymFormer forward on 8 TRN2 NeuronCores — data-parallel over batch.

Strategy:
 - B=8 -> one batch element per core, no collectives.
 - All *linear* stacks (relation encoder 26->256->256->128, joint encoder
   96->...->128, decoder 128->...->90) are identity-activation MLPs, so they
   fold on the host into single matmuls. All LayerNorm affine params and the
   0.25 attention scale also fold into adjacent weights.
 - Relation branch: per core compute attn_R for all 4 depths x 8 heads
   (256,256,32) straight into SBUF in k-major layout (k on partitions,
   q on free axis) so attention consumes it with zero transposes.
 - Softmax is computed k-on-partitions without max subtraction (logits are
   O(0.5) for this model); column sums via ones-vector matmuls on PE.
 - Matmuls run as float32r (1 cycle/row at N>=256 vs 4 for fp32).
"""

import sys

sys.path.insert(0, "/opt/trn_rl_repo")

import numpy as np

import concourse.bacc as bacc
import concourse.bass as bass
import concourse.mybir as mybir
import concourse.tile as tile
from concourse.bass_utils import run_bass_kernel_spmd

B, N, C, H, DP = 8, 256, 128, 8, 4
HS = C // H
SCALE = 0.25
NN = N * N          # 65536 relation rows per core
TT = 512            # relation rows per tile
NT = NN // TT       # 128 tiles
F32 = mybir.dt.float32
F32R = mybir.dt.float32r
AF = mybir.ActivationFunctionType
ALU = mybir.AluOpType

last_results = None  # BassKernelResults of the most recent run (for test.py)


def _r(ap):
    return ap.bitcast(F32R)


def _fold(inp):
    """Host-side weight folding. Returns dict of staged weight arrays."""
    f = lambda k: np.asarray(inp[k], np.float32)

    w = {}
    # relation encoder collapses to one 26->128 matmul
    Wc = f("re_w1") @ f("re_w2") @ f("re_w3")
    bc = (f("re_b1") @ f("re_w2") + f("re_b2")) @ f("re_w3") + f("re_b3")
    # relation branch fully folded:
    #   A.T = Wq_aug.T @ [x;1]  (26->32 direct, bias in augmented row)
    #   128*var = || Rc @ [x;1] ||^2  (Cholesky of the centered Gram)
    P = np.eye(128, dtype=np.float64) - 1.0 / 128.0
    Mh = np.concatenate([(P @ Wc.T.astype(np.float64)),
                         (P @ bc.astype(np.float64).reshape(128, 1))], axis=1)
    G = Mh.T @ Mh
    Rc = np.linalg.cholesky(G + 1e-14 * np.eye(27)).T  # upper: Rc.T@Rc = G
    RcT = np.zeros((91, 27), np.float32)
    RcT[0:27] = Rc.T.astype(np.float32)
    RcT[64:91] = Rc.T.astype(np.float32)
    w["RcT"] = RcT
    # ln2 + SCALE fold into rconv: per depth W'(128,8), b'(8)
    Wr = np.empty((128, DP, H), np.float32)
    br = np.empty((DP, H), np.float32)
    for i in range(DP):
        Wr[:, i, :] = SCALE * (f("ln2_g")[i][:, None] * f("rconv_w")[i])
        br[i] = SCALE * (f("ln2_b")[i] @ f("rconv_w")[i] + f("rconv_b")[i])
    Wr2 = Wr.reshape(128, DP * H)
    # fold mean-centering of the layernorm into the weight itself:
    # norm(x) @ W = (x @ (W - ones*colsum(W)/128)) * rs  (+ bias term)
    Wr2 = Wr2 - np.ones((128, 1), np.float32) * (Wr2.sum(0, keepdims=True) / 128.0)
    Wq = np.zeros((91, 32), np.float32)
    Wqa = np.concatenate([Wc @ Wr2, (Wr2.T @ bc).reshape(1, 32)], axis=0)  # (27,32)
    Wq[0:27] = Wqa
    Wq[64:91] = Wqa
    w["Wq"] = Wq
    w["o27"] = np.full((27, 32), 1.0 / 128.0, np.float32)
    brB4 = np.broadcast_to(br.reshape(-1), (128, 2, 2, 32))
    w["brB4"] = np.ascontiguousarray(brB4)
    # joint encoder collapse 96->128
    Wj = f("je_w1") @ f("je_w2") @ f("je_w3")
    bj = (f("je_b1") @ f("je_w2") + f("je_b2")) @ f("je_w3") + f("je_b3")
    w["Wj"] = np.ascontiguousarray(Wj)
    w["bj"] = np.ascontiguousarray(bj.reshape(128, 1))
    # per-depth block weights: ln1 folds into qkv (+ SCALE on q), ln3 into mw1
    qkvw = np.empty((DP, C, 3 * C), np.float32)
    qkvb = np.empty((DP, 3 * C), np.float32)
    mw1 = np.empty((DP, C, C), np.float32)
    mb1 = np.empty((DP, C), np.float32)
    for i in range(DP):
        qkvw[i] = f("ln1_g")[i][:, None] * f("qkv_w")[i]
        qkvb[i] = f("ln1_b")[i] @ f("qkv_w")[i] + f("qkv_b")[i]
        qkvw[i][:, :C] *= SCALE
        qkvb[i][:C] *= SCALE
        mw1[i] = f("ln3_g")[i][:, None] * f("mw1")[i]
        mb1[i] = f("ln3_b")[i] @ f("mw1")[i] + f("mb1")[i]
    # stage feature-major: (C_in, depth, ...).  q/k head channels are
    # zero-padded to 32-aligned partition bases: 3 groups of heads
    # (0,1,2) (3,4,5) (6,7) at bases 0/32/64 within each group tile.
    qkw = np.zeros((C, DP, 2, 3, C), np.float32)
    qkb = np.zeros((C, DP, 2, 3), np.float32)
    for i in range(DP):
        for t in range(2):          # 0=q, 1=k
            wt = qkvw[i][:, t * C : (t + 1) * C]   # (C_in, C)
            bt = qkvb[i][t * C : (t + 1) * C]
            for h in range(H):
                g, j = divmod(h, 3)
                qkw[:, i, t, g, 32 * j : 32 * j + HS] = wt[:, h * HS : (h + 1) * HS]
                qkb[32 * j : 32 * j + HS, i, t, g] = bt[h * HS : (h + 1) * HS]
    w["qkw"] = np.ascontiguousarray(qkw)
    w["qkb"] = np.ascontiguousarray(qkb)
    w["vw"] = np.ascontiguousarray(qkvw.transpose(1, 0, 2)[:, :, 2 * C :])
    w["vb"] = np.ascontiguousarray(qkvb[:, 2 * C :].T)
    projg = np.zeros((C, DP, 3, C), np.float32)   # padded group rows
    for i in range(DP):
        for h in range(H):
            g, j = divmod(h, 3)
            projg[32 * j : 32 * j + HS, i, g, :] = f("proj_w")[i][
                h * HS : (h + 1) * HS, :
            ]
    w["projw"] = np.ascontiguousarray(projg)
    w["projb"] = np.ascontiguousarray(f("proj_b").T)
    w["mw1"] = np.ascontiguousarray(mw1.transpose(1, 0, 2))
    w["mb1"] = np.ascontiguousarray(mb1.T)
    w["mw2"] = np.ascontiguousarray(f("mw2").transpose(1, 0, 2))
    w["mb2"] = np.ascontiguousarray(f("mb2").T)
    # decoder collapse with final LN affine folded in
    Wdc = f("dw1") @ f("dw2") @ f("dw3")
    Wd = f("ng")[:, None] * Wdc
    bd = f("nb") @ Wdc + (f("db1") @ f("dw2") + f("db2")) @ f("dw3") + f("db3")
    w["Wd"] = np.ascontiguousarray(Wd)
    w["bd"] = np.ascontiguousarray(bd.reshape(90, 1))
    # constants
    w["ident"] = np.eye(128, dtype=np.float32)
    w["identr"] = w["ident"]
    w["ones16"] = np.ones((1, 16), np.float32)
    w["ones32"] = np.ones((1, 32), np.float32)
    w["omean"] = np.full((128, 1), 1.0 / 128.0, np.float32)
    return w


PHASES = (1, 2)  # debug: which phases to emit
DP_EMIT = DP     # debug: how many transformer depths to emit
P2_LVL = 9      # debug: 0=LN+qkv only, 1=+pairs, 2=+norm, 3=+proj, 9=all
P2_SUB = 9      # debug: 0=no R-add/conn, 1=+R-add, 2=+conn-mul


def _build():
    nc = bacc.Bacc(None, target_bir_lowering=False)
    dram = {}

    def din(name, shape, dt=F32):
        dram[name] = nc.dram_tensor(name, list(shape), dt, kind="ExternalInput")
        return dram[name]

    rel4 = din("rel4", (16, 91, NN // 32), F32R)
    connT = din("connT", (128, 2, 2, N))
    jT = din("jT", (96, N), F32R)
    RcT = din("RcT", (91, 27), F32R)
    Wq = din("Wq", (91, 32), F32R)
    o27 = din("o27", (27, 32), F32R)
    brB4 = din("brB4", (128, 2, 2, 32))
    Wj = din("Wj", (96, 128), F32R)
    bj = din("bj", (128, 1))
    qkw = din("qkw", (C, DP, 2, 3, C), F32R)
    qkb = din("qkb", (C, DP, 2, 3))
    vw = din("vw", (C, DP, C), F32R)
    vb = din("vb", (C, DP))
    projw = din("projw", (C, DP, 3, C), F32R)
    projb = din("projb", (C, DP))
    mw1 = din("mw1", (C, DP, C), F32R)
    mb1 = din("mb1", (C, DP))
    mw2 = din("mw2", (C, DP, C), F32R)
    mb2 = din("mb2", (C, DP))
    Wd = din("Wd", (128, 90), F32R)
    bd = din("bd", (90, 1))
    ident = din("ident", (128, 128))
    identr = din("identr", (128, 128), F32R)
    ones16 = din("ones16", (1, 16), F32R)
    out_d = nc.dram_tensor("out", [N, 90], F32, kind="ExternalOutput")

    from contextlib import ExitStack

    with tile.TileContext(nc) as tc, ExitStack() as ctx, nc.allow_low_precision(
        reason="f32r matmul pipeline; end-to-end precision checked in test"
    ):
        const = ctx.enter_context(tc.tile_pool(name="const", bufs=1))
        zin = ctx.enter_context(tc.tile_pool(name="zin", bufs=2))
        zsb = ctx.enter_context(tc.tile_pool(name="zsb", bufs=3))
        st = ctx.enter_context(tc.tile_pool(name="st", bufs=4))
        psm = ctx.enter_context(tc.tile_pool(name="psm", bufs=8, space="PSUM"))
        pbig = psm
        pz = pbig
        ptr = psm
        pa = psm
        p2 = pbig
        p2b = psm
        pop = pbig
        wrk = ctx.enter_context(tc.tile_pool(name="wrk", bufs=2))
        wrk4 = ctx.enter_context(tc.tile_pool(name="wrk4", bufs=6))

        def cload(dt_handle, shape, tag, dt=F32):
            t = const.tile(list(shape), dt, tag=tag)
            nc.sync.dma_start(out=t, in_=dt_handle[:])
            return t

        RcT_s = cload(RcT, (91, 27), "RcT", F32R)
        Wq_s = cload(Wq, (91, 32), "Wq", F32R)
        o27_s = cload(o27, (27, 32), "o27", F32R)
        brB4_s = cload(brB4, (128, 2, 2, 32), "brB4")

        Wj_s = cload(Wj, (96, 128), "Wj", F32R)
        bj_s = cload(bj, (128, 1), "bj")
        qkw_s = cload(qkw, (C, DP, 2, 3, C), "qkw", F32R)
        qkb_s = cload(qkb, (C, DP, 2, 3), "qkb")
        vw_s = cload(vw, (C, DP, C), "vw", F32R)
        vb_s = cload(vb, (C, DP), "vb")
        projw_s = cload(projw, (C, DP, 3, C), "projw", F32R)
        projb_s = cload(projb, (C, DP), "projb")
        mw1_s = cload(mw1, (C, DP, C), "mw1", F32R)
        mb1_s = cload(mb1, (C, DP), "mb1")
        mw2_s = cload(mw2, (C, DP, C), "mw2", F32R)
        mb2_s = cload(mb2, (C, DP), "mb2")
        Wd_s = cload(Wd, (128, 90), "Wd", F32R)
        bd_s = cload(bd, (90, 1), "bd")
        id_s = cload(ident, (128, 128), "ident")
        idr_s = cload(identr, (128, 128), "identr", F32R)
        o16_s = cload(ones16, (1, 16), "ones16", F32R)
        conn_s = cload(connT, (128, 2, 2, N), "connT")
        eps_s = const.tile([128, 1], F32, tag="eps")
        nc.vector.memset(eps_s[:], 1e-5)
        jT_s = cload(jT, (96, N), "jT", F32R)

        R_T = const.tile([128, 2, N, 32], F32, tag="R_T")

        # ---------------- Phase 1: relation branch -> R_T ----------------
        def p1_mms(t, relq):
            g, j = divmod(t % 8, 4)
            xs = relq[64 * g : 64 * g + 27, j * TT : (j + 1) * TT]
            yc_ps = psm.tile([27, TT], F32, tag="b")
            nc.tensor.matmul(
                yc_ps, RcT_s[64 * g : 64 * g + 27, :], xs, start=True, stop=True
            )
            a_ps = psm.tile([32, TT], F32, tag="b")
            nc.tensor.matmul(
                a_ps, Wq_s[64 * g : 64 * g + 27, :], xs, start=True, stop=True
            )
            return yc_ps, a_ps

        def p1_rest(t, yc_ps, a_ps):
            ycsq = zsb.tile([27, TT], F32R, tag="ycsq")
            nc.scalar.activation(ycsq, yc_ps, AF.Square)
            var_ps = psm.tile([32, TT], F32, tag="b")
            nc.tensor.matmul(var_ps, o27_s[:], ycsq[:], start=True, stop=True)
            sg32 = zsb.tile([32, TT], F32, tag="sg32")
            nc.scalar.activation(sg32, var_ps, AF.Sqrt, bias=eps_s[0:32, :])
            a_sb = zsb.tile([32, TT], F32R, tag="a_sb")
            nc.vector.tensor_tensor(
                out=a_sb[:], in0=a_ps[:], in1=sg32[:], op=ALU.divide
            )
            r4p = psm.tile([128, 2, 2, 32], F32R, tag="b")
            for s in range(4):
                nc.tensor.transpose(
                    r4p[:, s % 2, s // 2, :],
                    a_sb[:, s * 128 : (s + 1) * 128],
                    idr_s[:32, :32],
                )
            nc.vector.tensor_add(
                out=R_T[:, :, 2 * t : 2 * t + 2, :], in0=r4p[:], in1=brB4_s[:]
            )

        def p1_mid(t, yc_ps, a_ps):
            ycsq = zsb.tile([27, TT], F32R, tag="ycsq")
            nc.scalar.activation(ycsq, yc_ps, AF.Square)
            var_ps = psm.tile([32, TT], F32, tag="b")
            nc.tensor.matmul(var_ps, o27_s[:], ycsq[:], start=True, stop=True)
            sg32 = zsb.tile([32, TT], F32, tag="sg32")
            nc.scalar.activation(sg32, var_ps, AF.Sqrt, bias=eps_s[0:32, :])
            rsb32 = zsb.tile([32, TT], F32, tag="rsb32")
            nc.vector.reciprocal(out=rsb32[:], in_=sg32[:])
            a_sb = zsb.tile([32, TT], F32R, tag="a_sb")
            nc.vector.tensor_mul(out=a_sb[:], in0=a_ps[:], in1=rsb32[:])
            return a_sb

        def p1_tail(t, a_sb):
            r4p = psm.tile([128, 2, 2, 32], F32R, tag="b")
            for s in range(4):
                nc.tensor.transpose(
                    r4p[:, s % 2, s // 2, :],
                    a_sb[:, s * 128 : (s + 1) * 128],
                    idr_s[:32, :32],
                )
            nc.vector.tensor_add(
                out=R_T[:, :, 2 * t : 2 * t + 2, :], in0=r4p[:], in1=brB4_s[:]
            )

        st1, st2 = [], []
        for Q in (range(16) if 1 in PHASES else []):
            relq = zin.tile([91, NN // 32], F32R, tag="relq")
            nc.sync.dma_start(out=relq, in_=rel4[Q])
            for idx in range(8):
                t = Q * 8 + idx
                st1.append((t, *p1_mms(t, relq)))
                if len(st1) > 1:
                    tt, yc, ap = st1.pop(0)
                    st2.append((tt, p1_mid(tt, yc, ap)))
                if len(st2) > 1:
                    p1_tail(*st2.pop(0))
        for tt, yc, ap in st1:
            st2.append((tt, p1_mid(tt, yc, ap)))
        for args in st2:
            p1_tail(*args)

        # ---------------- joint encoder -> jf (token-major, 2 tiles) -----
        jf = []
        for qt in range(2):
            jft = const.tile([128, 128], F32, tag=f"jf{qt}")
            jf.append(jft)
        jp = p2.tile([128, N], F32, tag="b")
        nc.tensor.matmul(jp, _r(Wj_s[:]), _r(jT_s[:]), start=True, stop=True)
        jfT = wrk.tile([128, N], F32, tag="jfT")
        nc.scalar.activation(jfT, jp, AF.Identity, bias=bj_s[:])
        for qt in range(2):
            tp = ptr.tile([128, 128], F32, tag="b")
            nc.tensor.transpose(tp, jfT[:, qt * 128 : (qt + 1) * 128], id_s[:])
            nc.vector.tensor_copy(out=jf[qt][:], in_=tp[:])

        def layer_norm_t(src_tiles, tag):
            """Token-major standardize; returns feature-major (128, 256) tile."""
            xT = wrk.tile([128, N], F32R, tag="xT")
            for qt in range(2):
                st6 = st.tile([128, 6], F32, tag="st6")
                nc.vector.bn_stats(out=st6, in_=src_tiles[qt][:])
                mv = st.tile([128, 2], F32, tag="mv")
                nc.vector.bn_aggr(out=mv, in_=st6[:])
                sg1 = st.tile([128, 1], F32, tag="sg")
                nc.scalar.activation(sg1, mv[:, 1:2], AF.Sqrt, bias=eps_s[:])
                rs1 = st.tile([128, 1], F32, tag="rs")
                nc.vector.reciprocal(out=rs1, in_=sg1[:])
                xh = wrk4.tile([128, 128], F32, tag="xh")
                nc.vector.tensor_scalar(
                    out=xh, in0=src_tiles[qt][:], scalar1=mv[:, 0:1],
                    scalar2=rs1[:], op0=ALU.subtract, op1=ALU.mult,
                )
                tp = ptr.tile([128, 128], F32, tag="b")
                nc.tensor.transpose(tp, xh[:], id_s[:])
                nc.scalar.activation(
                    xT[:, qt * 128 : (qt + 1) * 128], tp, AF.Copy
                )
            return xT

        # ---------------- Phase 2: 4 transformer blocks -------------------
        for i in (range(DP)[:DP_EMIT] if 2 in PHASES else []):
            xT = layer_norm_t(jf, f"ln1_{i}")
            qkT = []      # [t][g] -> (128, 256) padded head-group tiles
            for t in range(2):
                row = []
                for g in range(0, 3, 2):
                    ng = min(2, 3 - g)
                    ps = psm.tile([128, 2, N], F32, tag="b")
                    for gg in range(ng):
                        nc.tensor.matmul(
                            ps[:, gg, :], _r(qkw_s[:, i, t, g + gg, :]), _r(xT[:]),
                            start=True, stop=True,
                        )
                    sb = wrk.tile([128, 2, N], F32R, tag=f"qk{t}{g}")
                    nc.scalar.activation(
                        sb, ps, AF.Identity,
                        bias=qkb_s[:, i, t, g : g + 1],
                    )
                    row.append(sb)
                qkT.append(row)

            def qk(t, g):
                return qkT[t][g // 2][:, g % 2, :]

            vps = psm.tile([128, N], F32, tag="b")
            nc.tensor.matmul(vps, _r(vw_s[:, i, :]), _r(xT[:]), start=True, stop=True)
            vT = wrk.tile([128, N], F32, tag="vT")
            nc.scalar.activation(vT, vps, AF.Identity, bias=vb_s[:, i : i + 1])

            v_ext = []
            for kt in range(2):
                vtp = psm.tile([128, 128], F32, tag="b")
                nc.tensor.transpose(vtp, vT[:, kt * 128 : (kt + 1) * 128], id_s[:])
                vx = wrk4.tile([128, H, 33], F32R, tag="vx")
                nc.vector.memset(vx[:, :, 16:32].bitcast(F32), 0.0)
                nc.vector.memset(vx[:, :, 32:33].bitcast(F32), 1.0)
                nc.vector.tensor_copy(
                    out=vx[:, :, 0:16], in_=vtp.rearrange("p (h c) -> p h c", h=H)
                )
                v_ext.append(vx)

            oTn_g = []
            for g in range(3):
                og = wrk.tile([128, N], F32R, tag=f"og{g}")
                nc.gpsimd.memset((og[96:128, :] if g < 2 else og[64:128, :]).bitcast(F32), 0.0)
                oTn_g.append(og)

            # per-head attention, software-pipelined: head h's matmul work
            # overlaps head h-1's normalization chain
            def eval_head(h):
                ih = i * 8 + h
                g, jj = divmod(h, 3)
                hp = slice(32 * jj, 32 * jj + HS)
                o_ps = psm.tile([33, N], F32, tag="b")
                sps = []
                for kt in range(2):
                    sp = psm.tile([128, N], F32, tag="b")
                    nc.tensor.matmul(
                        sp, _r(qk(1, g)[hp, kt * 128 : (kt + 1) * 128]),
                        _r(qk(0, g)[hp, :]), start=True, stop=True,
                    )
                    sps.append(sp)
                Eks = []
                for kt in range(2):
                    pl = wrk4.tile([128, N], F32, tag="pl")
                    nc.vector.tensor_add(
                        out=pl[:], in0=sps[kt][:], in1=R_T[:, kt, :, ih]
                    )
                    nc.gpsimd.tensor_mul(
                        out=pl[:], in0=pl[:], in1=conn_s[:, kt, 0, :]
                    )
                    Ek = wrk4.tile([128, N], F32R, tag="Ek")
                    nc.scalar.activation(Ek, pl[:], AF.Exp)
                    Eks.append(Ek)
                for kt in range(2):
                    nc.tensor.matmul(
                        o_ps, _r(v_ext[kt][:, h, :]), _r(Eks[kt][:]),
                        start=(kt == 0), stop=(kt == 1),
                    )
                return o_ps

            def norm_h(h, o_ps):
                g, jj = divmod(h, 3)
                hp = slice(32 * jj, 32 * jj + HS)
                dv = st.tile([1, N], F32R, tag="dv")
                nc.vector.reciprocal(out=dv[:], in_=o_ps[32:33, :])
                dh_ps = psm.tile([16, N], F32, tag="b")
                nc.tensor.matmul(dh_ps, _r(o16_s[:]), _r(dv[:]), start=True, stop=True)
                dh = wrk4.tile([16, N], F32, tag="dh")
                nc.scalar.activation(dh, dh_ps, AF.Identity)
                nc.vector.tensor_mul(
                    out=oTn_g[g][hp, :], in0=o_ps[0:16, :], in1=dh[:]
                )

            hqueue = []
            for h in range(H):
                hqueue.append((h, eval_head(h)))
                if len(hqueue) > 2:
                    norm_h(*hqueue.pop(0))
            while hqueue:
                norm_h(*hqueue.pop(0))
            pr_ps = psm.tile([128, N], F32, tag="b")
            for g in (range(3) if P2_LVL >= 3 else []):
                nc.tensor.matmul(
                    pr_ps, _r(projw_s[:, i, g, :]), _r(oTn_g[g][:]),
                    start=(g == 0), stop=(g == 2),
                )
            if P2_LVL >= 3:
                prT = wrk.tile([128, N], F32, tag="prT")
                nc.scalar.activation(
                    prT, pr_ps, AF.Identity, bias=projb_s[:, i : i + 1]
                )
                for qt in range(2):
                    tp = psm.tile([128, 128], F32, tag="b")
                    nc.tensor.transpose(tp, prT[:, qt * 128 : (qt + 1) * 128], id_s[:])
                    nc.vector.tensor_add(out=jf[qt][:], in0=jf[qt][:], in1=tp[:])

            xT3 = layer_norm_t(jf, f"ln3_{i}")
            h1p = p2.tile([128, N], F32, tag="b")
            nc.tensor.matmul(
                h1p, _r(mw1_s[:, i, :]), _r(xT3[:]), start=True, stop=True
            )
            h1 = wrk.tile([128, N], F32R, tag="h1")
            nc.scalar.activation(h1, h1p, AF.Gelu, bias=mb1_s[:, i : i + 1])
            h2p = p2.tile([128, N], F32, tag="b")
            nc.tensor.matmul(
                h2p, _r(mw2_s[:, i, :]), _r(h1[:]), start=True, stop=True
            )
            h2 = wrk.tile([128, N], F32, tag="h2")
            nc.scalar.activation(h2, h2p, AF.Identity, bias=mb2_s[:, i : i + 1])
            for qt in range(2):
                tp = ptr.tile([128, 128], F32, tag="b")
                nc.tensor.transpose(tp, h2[:, qt * 128 : (qt + 1) * 128], id_s[:])
                nc.vector.tensor_add(out=jf[qt][:], in0=jf[qt][:], in1=tp[:])

        # ---------------- decoder ----------------------------------------
        xTf = layer_norm_t(jf, "lnf")
        op_ps = p2.tile([90, N], F32, tag="b")
        nc.tensor.matmul(op_ps, _r(Wd_s[:]), _r(xTf[:]), start=True, stop=True)
        outT = wrk.tile([90, N], F32, tag="outT")
        nc.scalar.activation(outT, op_ps, AF.Identity, bias=bd_s[:])
        for qt in range(2):
            tp = ptr.tile([128, 90], F32, tag="b")
            nc.tensor.transpose(tp, outT[:, qt * 128 : (qt + 1) * 128], id_s[:90, :90])
            of = wrk4.tile([128, 90], F32, tag="of")
            nc.scalar.activation(of, tp, AF.Copy)
            nc.sync.dma_start(out=out_d[qt * 128 : (qt + 1) * 128, :], in_=of[:])

    nc.compile()
    return nc


def kernel(**inputs):
    global last_results
    w = _fold(inputs)
    rel = np.asarray(inputs["relation_in"], np.float32)
    conn = np.asarray(inputs["conn"], np.float32)
    joint = np.asarray(inputs["joint_in"], np.float32)

    in_maps = []
    for b in range(B):
        m = dict(w)
        rT = rel[b].reshape(NN, 26).T          # (26, NN)
        r16 = np.zeros((16, 91, NN // 32), np.float32)
        for Q in range(16):
            r16[Q, 0:26] = rT[:, Q * 4096 : Q * 4096 + 2048]
            r16[Q, 26] = 1.0
            r16[Q, 64:90] = rT[:, Q * 4096 + 2048 : (Q + 1) * 4096]
            r16[Q, 90] = 1.0
        m["rel4"] = r16
        cT = conn[b].T.reshape(2, 128, N).transpose(1, 0, 2)  # (128, kt, q)
        m["connT"] = np.ascontiguousarray(
            np.broadcast_to(cT[:, :, None, :], (128, 2, 2, N)).copy()
        )
        m["jT"] = np.ascontiguousarray(joint[b].T)
        in_maps.append(m)

    nc = _build()
    last_results = run_bass_kernel_spmd(nc, in_maps, core_ids=list(range(B)))
    out = np.stack([r["out"] for r in last_results.results])
    return out.astype(np.float32)





# revision 24
# speedup vs baseline: 2.0735x; 2.0735x over previous
"""AsymFormer forward on 8 TRN2 NeuronCores — data-parallel over batch.

Strategy (v2):
 - B=8 -> one batch element per core, no collectives.
 - Host folds: relation encoder (26->...->128) + LN2 + rconv collapse into a
   single (27, 59) bf16 matrix [Rc^T | Wq]: per relation row, yc = Rc x_hat
   gives the LN variance via ||yc||^2 (Cholesky of the centered Gram, scaled
   so sumsq == var), and a = x_hat @ Wq gives the 32 per-depth-head relation
   logits pre-normalization.
 - Phase 1 runs x-STATIONARY: each matmul takes a (27,128) slice of the
   relation tensor as the stationary operand, so outputs land rows-on-
   partitions (128 relation rows x 59 cols). Variance -> Square(Act) ->
   grouped reduce(DVE) -> rsqrt via exp(-0.5*ln(v+eps)) (Act, single table)
   -> one broadcast multiply per 1024 rows writes R directly in the k-major
   layout attention consumes. No transposes, no wide low-partition ops.
 - Single activation table (natural_log_exp_and_others): Exp/Ln/Square/
   Identity/Copy only. LN rsqrt = exp(-0.5 ln(v+eps)); GELU = x*sigmoid
   (1.702x) computed from Exp + divide.
 - Attention: R added into score PSUM via identity matmul; conn multiply on
   DVE/Pool; softmax denominator rides the attn.v matmul as a ones column,
   packed 4 heads per PSUM tile so one strided reciprocal serves 4 heads.
 - bf16 everywhere on matmul operands (1 cycle/row incl. sub-256 free dims).
"""

import sys

sys.path.insert(0, "/opt/trn_rl_repo")

import numpy as np

import concourse.bacc as bacc
import concourse.bass as bass
import concourse.mybir as mybir
import concourse.tile as tile
from concourse.bass_utils import run_bass_kernel_spmd

B, N, C, H, DP = 8, 256, 128, 8, 4
HS = C // H
SCALE = 0.25
NN = N * N
F32 = mybir.dt.float32
BF16 = mybir.dt.bfloat16
AF = mybir.ActivationFunctionType
ALU = mybir.AluOpType

last_results = None  # BassKernelResults of the most recent run (for test.py)

try:
    import ml_dtypes

    _BF = ml_dtypes.bfloat16
except ImportError:  # pragma: no cover
    _BF = np.float32


def _bf(a):
    return np.ascontiguousarray(np.asarray(a, np.float32)).astype(_BF)


def _fold(inp):
    """Host-side weight folding. Returns dict of staged weight arrays."""
    f = lambda k: np.asarray(inp[k], np.float32)

    w = {}
    # relation encoder collapses to one 26->128 affine map
    Wc = f("re_w1") @ f("re_w2") @ f("re_w3")
    bc = (f("re_b1") @ f("re_w2") + f("re_b2")) @ f("re_w3") + f("re_b3")
    P = np.eye(128, dtype=np.float64) - 1.0 / 128.0
    Mh = np.concatenate(
        [P @ Wc.T.astype(np.float64), P @ bc.astype(np.float64).reshape(128, 1)],
        axis=1,
    )
    G = Mh.T @ Mh / 128.0  # scaled so x^T G x == var
    Rc = np.linalg.cholesky(G + 1e-16 * np.eye(27)).T  # upper: Rc^T Rc = G
    # ln2 + SCALE fold into rconv: per depth W'(128,8)
    Wr = np.empty((128, DP, H), np.float32)
    br = np.empty((DP, H), np.float32)
    for i in range(DP):
        Wr[:, i, :] = SCALE * (f("ln2_g")[i][:, None] * f("rconv_w")[i])
        br[i] = SCALE * (f("ln2_b")[i] @ f("rconv_w")[i] + f("rconv_b")[i])
    Wr2 = Wr.reshape(128, DP * H)
    # mean-centering of the layernorm folds into the weight itself
    Wr2 = Wr2 - np.ones((128, 1), np.float32) * (Wr2.sum(0, keepdims=True) / 128.0)
    Wqa = np.concatenate([Wc @ Wr2, (Wr2.T @ bc).reshape(1, 32)], axis=0)  # (27,32)
    w["W1"] = _bf(np.concatenate([Rc.T.astype(np.float32), Wqa], axis=1))  # (27,59)

    # joint encoder collapse 96->128
    Wj = f("je_w1") @ f("je_w2") @ f("je_w3")
    bj = (f("je_b1") @ f("je_w2") + f("je_b2")) @ f("je_w3") + f("je_b3")
    w["Wj"] = _bf(Wj)
    w["bj"] = np.ascontiguousarray(bj.reshape(128, 1))

    # per-depth block weights: ln1 folds into qkv (+ SCALE on q), ln3 into mw1
    qkvw = np.empty((DP, C, 3 * C), np.float32)
    qkvb = np.empty((DP, 3 * C), np.float32)
    mw1 = np.empty((DP, C, C), np.float32)
    mb1 = np.empty((DP, C), np.float32)
    for i in range(DP):
        qkvw[i] = f("ln1_g")[i][:, None] * f("qkv_w")[i]
        qkvb[i] = f("ln1_b")[i] @ f("qkv_w")[i] + f("qkv_b")[i]
        qkvw[i][:, :C] *= SCALE
        qkvb[i][:C] *= SCALE
        mw1[i] = f("ln3_g")[i][:, None] * f("mw1")[i]
        mb1[i] = f("ln3_b")[i] @ f("mw1")[i] + f("mb1")[i]
    # q/k head channels zero-padded to 32-aligned bases: head h -> group
    # g=h//3, slot j=h%3 at rows 32j..32j+16 of group tile; row 32j+16 is the
    # pad row used for the relation bias: q-pad=1, k-pad=br.
    qkw = np.zeros((C, DP, 2, 3, C), np.float32)
    qkbias = np.zeros((1, DP, 2, 3, C), np.float32)
    for i in range(DP):
        for t in range(2):
            wt = qkvw[i][:, t * C : (t + 1) * C]
            bt = qkvb[i][t * C : (t + 1) * C]
            for h in range(H):
                g, j = divmod(h, 3)
                qkw[:, i, t, g, 32 * j : 32 * j + HS] = wt[:, h * HS : (h + 1) * HS]
                qkbias[0, i, t, g, 32 * j : 32 * j + HS] = bt[h * HS : (h + 1) * HS]
                qkbias[0, i, t, g, 32 * j + HS] = 1.0 if t == 0 else br[i, h]
    w["qkw"] = _bf(qkw)
    w["qkbias"] = _bf(qkbias)
    w["vw"] = _bf(qkvw.transpose(1, 0, 2)[:, :, 2 * C :])  # (C, DP, C)
    w["vb_row"] = _bf(qkvb[:, 2 * C :].reshape(1, DP, C))
    projp = np.zeros((C, DP, 2, C), np.float32)
    for i in range(DP):
        for h in range(H):
            qq, m = divmod(h, 4)
            projp[32 * m : 32 * m + HS, i, qq, :] = f("proj_w")[i][
                h * HS : (h + 1) * HS, :
            ]
    w["projw"] = _bf(projp)  # (C, DP, 2, C), rows 32m+d of quad qq
    w["projb"] = np.ascontiguousarray(f("proj_b").T)  # (C, DP)
    w["mw1"] = _bf(mw1.transpose(1, 0, 2))
    w["mb1"] = np.ascontiguousarray(mb1.T)
    w["mw2"] = _bf(f("mw2").transpose(1, 0, 2))
    w["mb2"] = np.ascontiguousarray(f("mb2").T)

    # decoder collapse with final LN affine folded in
    Wdc = f("dw1") @ f("dw2") @ f("dw3")
    Wd = f("ng")[:, None] * Wdc
    bd = f("nb") @ Wdc + (f("db1") @ f("dw2") + f("db2")) @ f("dw3") + f("db3")
    w["Wd"] = _bf(Wd)
    w["bd"] = np.ascontiguousarray(bd.reshape(90, 1))

    # constants
    w["idr16"] = _bf(np.eye(128, dtype=np.float32))
    w["id32"] = np.eye(128, dtype=np.float32)
    w["o16"] = _bf(np.ones((1, 16), np.float32))
    # denominator-broadcast selectors: sel4[m, p2, c]: row m of pair-block
    # p2 -> cols 32*(2*p2+m)..+16
    sel = np.zeros((2, 2, 128), np.float32)
    for p2 in range(2):
        for m in range(2):
            sel[m, p2, 32 * (2 * p2 + m) : 32 * (2 * p2 + m) + 16] = 1.0
    w["sel4"] = _bf(sel)
    return w


NSUP = 32          # phase-1 tiles (2048 relation rows each)
G4 = 2             # groups of 1024 rows per tile
DP_EMIT = DP       # debug knob


def _build():
    nc = bacc.Bacc(None, target_bir_lowering=False)
    dram = {}

    def din(name, shape, dt=BF16):
        dram[name] = nc.dram_tensor(name, list(shape), dt, kind="ExternalInput")
        return dram[name]

    relT = din("relT", (NSUP, 27, 2048))
    connT = din("connT", (128, 2, N))
    jT = din("jT", (96, N))
    W1 = din("W1", (27, 59))
    Wj = din("Wj", (96, 128))
    bj = din("bj", (128, 1), F32)
    qkw = din("qkw", (C, DP, 2, 3, C))
    qkbias = din("qkbias", (1, DP, 2, 3, C))
    vw = din("vw", (C, DP, C))
    vb_row = din("vb_row", (1, DP, C))
    projw = din("projw", (C, DP, 2, C))
    projb = din("projb", (C, DP), F32)
    mw1 = din("mw1", (C, DP, C))
    mb1 = din("mb1", (C, DP), F32)
    mw2 = din("mw2", (C, DP, C))
    mb2 = din("mb2", (C, DP), F32)
    Wd = din("Wd", (128, 90))
    bd = din("bd", (90, 1), F32)
    idr16 = din("idr16", (128, 128))
    id32 = din("id32", (128, 128), F32)
    o16 = din("o16", (1, 16))
    sel4 = din("sel4", (2, 2, 128))
    out_d = nc.dram_tensor("out", [N, 90], F32, kind="ExternalOutput")

    from contextlib import ExitStack

    with tile.TileContext(nc) as tc, ExitStack() as ctx, nc.allow_low_precision(
        reason="bf16 matmul pipeline; end-to-end precision checked in test"
    ):
        const = ctx.enter_context(tc.tile_pool(name="const", bufs=1))

        # Pre-load the one activation table (natural_log_exp_and_others)
        # covering Ln/Exp/Square/Identity/Copy — the automatic inserter
        # would otherwise thrash between the natural_log and exp tables
        # on every LN rsqrt (1283ns per load).
        nc.scalar.add_instruction(
            mybir.InstLoadActFuncSet(
                name=nc.get_next_instruction_name(),
                ins=[],
                outs=[],
                act_func_set_id=6,
            )
        )

        def cload(h, shape, tag, dt=BF16):
            t = const.tile(list(shape), dt, tag=tag)
            nc.sync.dma_start(out=t, in_=h[:])
            return t

        W1_s = cload(W1, (27, 59), "W1")
        jT_s = cload(jT, (96, N), "jT")
        Wj_s = cload(Wj, (96, 128), "Wj")
        bj_s = cload(bj, (128, 1), "bj", F32)
        id32_s = cload(id32, (128, 128), "id32", F32)
        ones_s = const.tile([1, N], BF16, tag="ones_s")
        nc.vector.memset(ones_s[:], 1.0)
        eps_s = const.tile([128, 1], F32, tag="eps")
        nc.vector.memset(eps_s[:], 1e-5)
        # R[k, q, kt, ch]: relation logits (a * rsqrt(var)) in bf16
        R2 = const.tile([128, N, 2, 32], BF16, tag="R2")
        oTn = [
            const.tile([128, N], BF16, tag=f"oTn{qq}", name=f"oTn{qq}")
            for qq in range(2)
        ]
        for qq in range(2):
            nc.vector.memset(oTn[qq][:], 0.0)
        jf = [
            const.tile([128, 128], F32, tag=f"jf{qt}", name=f"jf{qt}")
            for qt in range(2)
        ]

        # ---------------- joint encoder -> jf (token-major) ---------------
        with tc.tile_pool(name="jenc", bufs=1, space="PSUM") as jpool:
            jp = jpool.tile([128, N], F32, tag="jp")
            nc.tensor.matmul(jp, Wj_s[:], jT_s[:], start=True, stop=True)
            jfT = const.tile([128, N], F32, tag="jfT")
            nc.scalar.activation(jfT, jp, AF.Identity, bias=bj_s[:])
            for qt in range(2):
                tp = jpool.tile([128, 128], F32, tag="jtp")
                nc.tensor.transpose(tp, jfT[:, qt * 128 : (qt + 1) * 128], id32_s[:])
                nc.vector.tensor_copy(out=jf[qt][:], in_=tp[:])

        # ---------------- Phase 1: relation branch -> R2 -------------------
        with tc.tile_pool(name="p1ps", bufs=2, space="PSUM") as pgp, \
             tc.tile_pool(name="p1sb", bufs=6) as p1sb, \
             tc.tile_pool(name="p1st", bufs=4) as p1st:
            pend = []
            mul_q = []  # deferred multiplies: (aa_tile, rsS, a, q0)
            sq = None
            nmul = 0

            def emit_muls():
                nonlocal nmul
                for aa_t, rs_t, a, q0 in mul_q:
                    eng = nc.vector
                    nmul += 1
                    eng.tensor_tensor(
                        out=R2[:, q0 : q0 + 8, :, :].rearrange(
                            "k (g q) kt c -> k g (q kt) c", g=G4
                        ),
                        in0=aa_t[:, :, :, :],
                        in1=rs_t[:, a, :, :, None].to_broadcast((128, G4, 8, 32)),
                        op=ALU.mult,
                    )
                mul_q.clear()

            for s in range(NSUP):
                relq = p1sb.tile([27, 2048], BF16, tag="relq")
                nc.sync.dma_start(out=relq, in_=relT[s])
                yc = pgp.tile([128, G4, 8, 32], F32, tag="yc", bufs=3)
                aa = pgp.tile([128, G4, 8, 32], F32, tag="aa", bufs=5)
                for g4 in range(G4):
                    for c in range(8):
                        k = g4 * 8 + c
                        xsl = relq[:, k * 128 : (k + 1) * 128]
                        nc.tensor.matmul(
                            yc[:, g4, c, 0:27], xsl, W1_s[:, 0:27],
                            start=True, stop=True,
                        )
                        nc.tensor.matmul(
                            aa[:, g4, c, :], xsl, W1_s[:, 27:59],
                            start=True, stop=True,
                        )
                if s % 2 == 0:
                    sq = p1sb.tile([128, 2, G4, 8, 27], BF16, tag="sq", bufs=3)
                nc.scalar.activation(sq[:, s % 2], yc[:, :, :, 0:27], AF.Square)
                pend.append(aa)
                if s % 2 == 1:
                    vS = p1st.tile([128, 2, G4, 8], F32, tag="vS")
                    nc.vector.tensor_reduce(
                        out=vS[:],
                        in_=sq[:],
                        axis=mybir.AxisListType.X,
                        op=ALU.add,
                    )
                    lnv = p1st.tile([128, 2, G4, 8], F32, tag="lnv")
                    nc.scalar.activation(
                        lnv.rearrange("p a g c -> p (a g c)"),
                        vS.rearrange("p a g c -> p (a g c)"),
                        AF.Ln,
                        bias=eps_s[:],
                    )
                    rsS = p1st.tile([128, 2, G4, 8], F32, tag="rsS")
                    nc.scalar.activation(
                        rsS.rearrange("p a g c -> p (a g c)"),
                        lnv.rearrange("p a g c -> p (a g c)"),
                        AF.Exp,
                        scale=-0.5,
                    )
                    nxt = [
                        (pga, rsS, a, (s - 1 + a) * 8)
                        for a, pga in enumerate(pend)
                    ]
                    emit_muls()
                    mul_q.extend(nxt)
                    pend = []
            emit_muls()


        # phase-2 weights: loaded behind the relation-tile DMAs so the first
        # relq arrives ~12us earlier
        qkw_s = cload(qkw, (C, DP, 2, 3, C), "qkw")
        qkb_s = cload(qkbias, (1, DP, 2, 3, C), "qkbias")
        vw_s = cload(vw, (C, DP, C), "vw")
        vbr_s = cload(vb_row, (1, DP, C), "vb_row")
        projw_s = cload(projw, (C, DP, 2, C), "projw")
        projb_s = cload(projb, (C, DP), "projb", F32)
        mw1_s = cload(mw1, (C, DP, C), "mw1")
        mb1_s = cload(mb1, (C, DP), "mb1", F32)
        mw2_s = cload(mw2, (C, DP, C), "mw2")
        mb2_s = cload(mb2, (C, DP), "mb2", F32)
        Wd_s = cload(Wd, (128, 90), "Wd")
        bd_s = cload(bd, (90, 1), "bd", F32)
        idr_s = cload(idr16, (128, 128), "idr16")
        o16_s = cload(o16, (1, 16), "o16")
        sel_s = cload(sel4, (2, 2, 128), "sel4")
        conn_s = cload(connT, (128, 2, N), "connT")

        # ---------------- Phase 2: 4 transformer blocks --------------------
        p2ps = ctx.enter_context(tc.tile_pool(name="p2ps", bufs=1, space="PSUM"))
        spps = ctx.enter_context(tc.tile_pool(name="spps", bufs=3, space="PSUM"))
        oqps = ctx.enter_context(tc.tile_pool(name="oqps", bufs=1, space="PSUM"))
        trps = ctx.enter_context(tc.tile_pool(name="trps", bufs=2, space="PSUM"))
        wrk = ctx.enter_context(tc.tile_pool(name="wrk", bufs=2))
        ekp = ctx.enter_context(tc.tile_pool(name="ekp", bufs=3))
        st = ctx.enter_context(tc.tile_pool(name="st", bufs=4))

        def layer_norm_t(tag):
            """Token-major standardize; returns feature-major (128,256) bf16.
            Both token halves advance stage-by-stage so no engine stream
            head-blocks on the other half."""
            xT = wrk.tile([128, N], BF16, tag=f"xT_{tag}")
            st6, mv, lnv1, rs1, xh, tp = [[None, None] for _ in range(6)]
            for qt in range(2):
                st6[qt] = st.tile([128, 6], F32, tag="st6", name="st6")
                nc.vector.bn_stats(out=st6[qt], in_=jf[qt][:])
            for qt in range(2):
                mv[qt] = st.tile([128, 2], F32, tag="mv", name="mv")
                nc.vector.bn_aggr(out=mv[qt], in_=st6[qt][:])
            for qt in range(2):
                lnv1[qt] = st.tile([128, 1], F32, tag="lnv1", name="lnv1")
                nc.scalar.activation(
                    lnv1[qt], mv[qt][:, 1:2], AF.Ln, bias=eps_s[:]
                )
                rs1[qt] = st.tile([128, 1], F32, tag="rs1", name="rs1")
                nc.scalar.activation(rs1[qt], lnv1[qt][:], AF.Exp, scale=-0.5)
            for qt in range(2):
                xh[qt] = st.tile([128, 128], BF16, tag="xh", name="xh")
                nc.vector.tensor_scalar(
                    out=xh[qt], in0=jf[qt][:], scalar1=mv[qt][:, 0:1],
                    scalar2=rs1[qt][:], op0=ALU.subtract, op1=ALU.mult,
                )
            for qt in range(2):
                tp[qt] = trps.tile([128, 128], BF16, tag="sm", name="lntp")
                nc.tensor.transpose(tp[qt], xh[qt][:], idr_s[:])
                nc.scalar.activation(
                    xT[:, qt * 128 : (qt + 1) * 128], tp[qt], AF.Copy
                )
            return xT

        for i in range(DP_EMIT):
            xT = layer_norm_t(f"ln1_{i}")
            # qkv
            qkT = []
            for t in range(2):
                qk_ps = p2ps.tile([128, 3, N], F32, tag="qk_ps")
                for g in range(3):
                    for half in range(2):
                        hs = slice(half * 128, (half + 1) * 128)
                        nc.tensor.matmul(
                            qk_ps[:, g, hs], qkb_s[:, i, t, g, :],
                            ones_s[:, hs], start=True, stop=False,
                        )
                        nc.tensor.matmul(
                            qk_ps[:, g, hs], qkw_s[:, i, t, g, :], xT[:, hs],
                            start=False, stop=True,
                        )
                qkt = wrk.tile([128, 3, N], BF16, tag=f"qkt{t}")
                nc.scalar.activation(qkt, qk_ps, AF.Copy)
                qkT.append(qkt)
            # v directly token-major: out (tok, ch) per kt half
            vx = wrk.tile([128, 2, H, 16], BF16, tag="vx")
            ones_c = wrk.tile([128, 1], BF16, tag="ones_c")
            nc.vector.memset(ones_c[:], 1.0)
            for kt in range(2):
                v_ps = trps.tile([128, 128], F32, tag="sm", name="v_ps")
                nc.tensor.matmul(
                    v_ps, ones_s[:, kt * 128 : kt * 128 + 128], vbr_s[:, i, :],
                    start=True, stop=False,
                )
                nc.tensor.matmul(
                    v_ps, xT[:, kt * 128 : (kt + 1) * 128], vw_s[:, i, :],
                    start=False, stop=True,
                )
                nc.vector.tensor_copy(
                    out=vx[:, kt, :, :],
                    in_=v_ps.rearrange("p (h d) -> p h d", h=H),
                )

            oq_tiles = {}
            dn_tiles = {}

            def eval_head(h):
                ih = i * 8 + h
                g, j = divmod(h, 3)
                hp = slice(32 * j, 32 * j + HS + 1)
                sp = spps.tile([128, 2, N], F32, tag="sp")
                for kt in range(2):
                    nc.tensor.matmul(
                        sp[:, kt, :],
                        qkT[1][hp, g, kt * 128 : (kt + 1) * 128],
                        qkT[0][hp, g, :],
                        start=True, stop=False,
                    )
                    nc.tensor.matmul(
                        sp[:, kt, :], idr_s[:], R2[:, :, kt, ih],
                        start=False, stop=True,
                    )
                nc.vector.tensor_tensor(
                    out=sp[:], in0=sp[:], in1=conn_s[:], op=ALU.mult
                )
                ek = ekp.tile([128, 2, N], BF16, tag="ek")
                nc.scalar.activation(ek, sp, AF.Exp)
                q4, m = divmod(h, 4)
                if m == 0:
                    oq_tiles[q4] = oqps.tile([128, N], F32, tag="oq", name="oq")
                    nc.vector.memset(oq_tiles[q4][:], 0.0)
                    dn_tiles[q4] = trps.tile(
                        [128, N], F32, tag="sm", name="dn_ps"
                    )
                oq = oq_tiles[q4]
                dn = dn_tiles[q4]
                for kt in range(2):
                    nc.tensor.matmul(
                        oq[32 * m : 32 * m + HS, :],
                        vx[:, kt, h, :],
                        ek[:, kt, :],
                        start=(kt == 0), stop=(kt == 1),
                        tile_position=(0, 32 * m),
                    )
                    nc.tensor.matmul(
                        dn[32 * m : 32 * m + 1, :],
                        ones_c[:],
                        ek[:, kt, :],
                        start=(kt == 0), stop=(kt == 1),
                        tile_position=(0, 32 * m),
                    )

            def norm_quad2(q4):
                oq = oq_tiles[q4]
                dn = dn_tiles[q4]
                oqsb = st.tile([128, N], BF16, tag="oqsb", name="oqsb")
                nc.scalar.activation(oqsb, oq, AF.Copy)
                dvv = [None] * 4
                for m in range(4):
                    dvv[m] = st.tile([1, N], BF16, tag="dvv", name="dvv")
                    nc.vector.reciprocal(
                        out=dvv[m][:], in_=dn[32 * m : 32 * m + 1, :]
                    )
                dh_ps = trps.tile([128, N], F32, tag="sm", name="dh_ps")
                for m in range(4):
                    nc.tensor.matmul(
                        dh_ps[32 * m : 32 * m + 16, :], o16_s[:], dvv[m][:],
                        start=True, stop=True, tile_position=(0, 32 * m),
                    )
                for m in range(4):
                    nc.vector.tensor_tensor(
                        out=oTn[q4][32 * m : 32 * m + 16, :],
                        in0=oqsb[32 * m : 32 * m + 16, :],
                        in1=dh_ps[32 * m : 32 * m + 16, :],
                        op=ALU.mult,
                    )

            prb = p2ps.tile([128, 3, N], F32, tag="qk_ps", name="prb")
            pr_ps = prb[:, 0, :]
            for h in range(H):
                eval_head(h)
                if h == 3:
                    norm_quad2(0)
                if h == 7:
                    norm_quad2(1)
                if h == 5:
                    nc.tensor.matmul(
                        pr_ps, projw_s[:, i, 0, :], oTn[0][:],
                        start=True, stop=False,
                    )
            nc.tensor.matmul(
                pr_ps, projw_s[:, i, 1, :], oTn[1][:],
                start=False, stop=True,
            )
            prT = wrk.tile([128, N], BF16, tag="prT")
            rtp = [None, None]
            for qt in range(2):
                nc.scalar.activation(
                    prT[:, qt * 128 : (qt + 1) * 128],
                    pr_ps[:, qt * 128 : (qt + 1) * 128],
                    AF.Identity, bias=projb_s[:, i : i + 1],
                )
            for qt in range(2):
                rtp[qt] = trps.tile([128, 128], BF16, tag="sm", name="rtp")
                nc.tensor.transpose(
                    rtp[qt], prT[:, qt * 128 : (qt + 1) * 128], idr_s[:]
                )
            for qt in range(2):
                nc.vector.tensor_tensor(
                    out=jf[qt][:], in0=jf[qt][:], in1=rtp[qt][:], op=ALU.add
                )

            # MLP
            xT3 = layer_norm_t(f"ln3_{i}")
            h1b = p2ps.tile([128, 3, N], F32, tag="qk_ps", name="h1b")
            h1_ps = h1b[:, 0, :]
            h2b = p2ps.tile([128, 3, N], F32, tag="qk_ps", name="h2b")
            h2_ps = h2b[:, 0, :]
            h1sb = wrk.tile([128, N], F32, tag="h1sb")
            e16 = wrk.tile([128, N], F32, tag="e16")
            ep = wrk.tile([128, N], F32, tag="ep")
            r32 = wrk.tile([128, N], F32, tag="r32")
            g16 = wrk.tile([128, N], BF16, tag="g16")
            h2T = wrk.tile([128, N], BF16, tag="h2T")
            # gelu(x) ~= x * sigmoid(1.5957691 x), two per-token-half chains
            # advanced stage-by-stage (no engine stream head-blocking)
            halves = [slice(0, 128), slice(128, 256)]
            for qt, hs in enumerate(halves):
                nc.tensor.matmul(
                    h1_ps[:, hs], mw1_s[:, i, :], xT3[:, hs],
                    start=True, stop=True,
                )
            for qt, hs in enumerate(halves):
                nc.scalar.activation(
                    h1sb[:, hs], h1_ps[:, hs], AF.Identity,
                    bias=mb1_s[:, i : i + 1],
                )
                nc.scalar.activation(
                    e16[:, hs], h1_ps[:, hs], AF.Exp, scale=-1.5957691
                )
            for qt, hs in enumerate(halves):
                nc.vector.tensor_scalar(
                    out=ep[:, hs], in0=e16[:, hs], scalar1=1.0, scalar2=None,
                    op0=ALU.add,
                )
            for qt, hs in enumerate(halves):
                nc.vector.reciprocal(out=r32[:, hs], in_=ep[:, hs])
            for qt, hs in enumerate(halves):
                nc.vector.tensor_tensor(
                    out=g16[:, hs], in0=h1sb[:, hs], in1=r32[:, hs],
                    op=ALU.mult,
                )
            for qt, hs in enumerate(halves):
                nc.tensor.matmul(
                    h2_ps[:, hs], mw2_s[:, i, :], g16[:, hs],
                    start=True, stop=True,
                )
            for qt, hs in enumerate(halves):
                nc.scalar.activation(
                    h2T[:, hs], h2_ps[:, hs], AF.Identity,
                    bias=mb2_s[:, i : i + 1],
                )
            mtp = [None, None]
            for qt, hs in enumerate(halves):
                mtp[qt] = trps.tile([128, 128], BF16, tag="sm", name="rtp")
                nc.tensor.transpose(mtp[qt], h2T[:, hs], idr_s[:])
            for qt in range(2):
                nc.vector.tensor_tensor(
                    out=jf[qt][:], in0=jf[qt][:], in1=mtp[qt][:], op=ALU.add
                )

        # ---------------- decoder ------------------------------------------
        xTf = layer_norm_t("lnf")
        opb = p2ps.tile([128, 3, N], F32, tag="qk_ps", name="opb")
        op_ps = opb[0:90, 0, :]
        nc.tensor.matmul(op_ps, Wd_s[:], xTf[:], start=True, stop=True)
        outT = wrk.tile([90, N], F32, tag="outT")
        nc.scalar.activation(outT, op_ps, AF.Identity, bias=bd_s[:])
        for qt in range(2):
            tp = trps.tile([128, 90], F32, tag="sm", name="otp")
            nc.tensor.transpose(
                tp, outT[:, qt * 128 : (qt + 1) * 128], id32_s[:90, :90]
            )
            of = wrk.tile([128, 90], F32, tag="of")
            nc.vector.tensor_copy(out=of[:], in_=tp[:])
            nc.sync.dma_start(out=out_d[qt * 128 : (qt + 1) * 128, :], in_=of[:])

    nc.compile()
    return nc


def kernel(**inputs):
    global last_results
    w = _fold(inputs)
    rel = np.asarray(inputs["relation_in"], np.float32)
    conn = np.asarray(inputs["conn"], np.float32)
    joint = np.asarray(inputs["joint_in"], np.float32)

    in_maps = []
    for b in range(B):
        m = dict(w)
        rT = np.empty((27, NN), np.float32)
        rT[0:26] = rel[b].reshape(NN, 26).T
        rT[26] = 1.0
        m["relT"] = np.ascontiguousarray(
            rT.reshape(27, NSUP, 2048).transpose(1, 0, 2)
        ).astype(_BF)
        # connT[k%128, k//128, q] = conn[b][q, k]
        cT = conn[b].T.reshape(2, 128, N).transpose(1, 0, 2)
        m["connT"] = _bf(cT)
        m["jT"] = _bf(joint[b].T)
        in_maps.append(m)

    nc = _build()
    last_results = run_bass_kernel_spmd(nc, in_maps, core_ids=list(range(B)))
    out = np.stack([r["out"] for r in last_results.results])
    return out.astype(np.float32)


# revision 28
# speedup vs baseline: 2.0882x; 1.0071x over previous
"""AsymFormer forward on 8 TRN2 NeuronCores — data-parallel over batch.

Strategy (v2):
 - B=8 -> one batch element per core, no collectives.
 - Host folds: relation encoder (26->...->128) + LN2 + rconv collapse into a
   single (27, 59) bf16 matrix [Rc^T | Wq]: per relation row, yc = Rc x_hat
   gives the LN variance via ||yc||^2 (Cholesky of the centered Gram, scaled
   so sumsq == var), and a = x_hat @ Wq gives the 32 per-depth-head relation
   logits pre-normalization.
 - Phase 1 runs x-STATIONARY: each matmul takes a (27,128) slice of the
   relation tensor as the stationary operand, so outputs land rows-on-
   partitions (128 relation rows x 59 cols). Variance -> Square(Act) ->
   grouped reduce(DVE) -> rsqrt via exp(-0.5*ln(v+eps)) (Act, single table)
   -> one broadcast multiply per 1024 rows writes R directly in the k-major
   layout attention consumes. No transposes, no wide low-partition ops.
 - Single activation table (natural_log_exp_and_others): Exp/Ln/Square/
   Identity/Copy only. LN rsqrt = exp(-0.5 ln(v+eps)); GELU = x*sigmoid
   (1.702x) computed from Exp + divide.
 - Attention: R added into score PSUM via identity matmul; conn multiply on
   DVE/Pool; softmax denominator rides the attn.v matmul as a ones column,
   packed 4 heads per PSUM tile so one strided reciprocal serves 4 heads.
 - bf16 everywhere on matmul operands (1 cycle/row incl. sub-256 free dims).
"""

import sys

sys.path.insert(0, "/opt/trn_rl_repo")

import numpy as np

import concourse.bacc as bacc
import concourse.bass as bass
import concourse.mybir as mybir
import concourse.tile as tile
from concourse.bass_utils import run_bass_kernel_spmd

B, N, C, H, DP = 8, 256, 128, 8, 4
HS = C // H
SCALE = 0.25
NN = N * N
F32 = mybir.dt.float32
BF16 = mybir.dt.bfloat16
AF = mybir.ActivationFunctionType
ALU = mybir.AluOpType

last_results = None  # BassKernelResults of the most recent run (for test.py)

try:
    import ml_dtypes

    _BF = ml_dtypes.bfloat16
except ImportError:  # pragma: no cover
    _BF = np.float32


def _bf(a):
    return np.ascontiguousarray(np.asarray(a, np.float32)).astype(_BF)


def _fold(inp):
    """Host-side weight folding. Returns dict of staged weight arrays."""
    f = lambda k: np.asarray(inp[k], np.float32)

    w = {}
    # relation encoder collapses to one 26->128 affine map
    Wc = f("re_w1") @ f("re_w2") @ f("re_w3")
    bc = (f("re_b1") @ f("re_w2") + f("re_b2")) @ f("re_w3") + f("re_b3")
    P = np.eye(128, dtype=np.float64) - 1.0 / 128.0
    Mh = np.concatenate(
        [P @ Wc.T.astype(np.float64), P @ bc.astype(np.float64).reshape(128, 1)],
        axis=1,
    )
    G = Mh.T @ Mh / 128.0  # scaled so x^T G x == var
    Rc = np.linalg.cholesky(G + 1e-16 * np.eye(27)).T  # upper: Rc^T Rc = G
    # ln2 + SCALE fold into rconv: per depth W'(128,8)
    Wr = np.empty((128, DP, H), np.float32)
    br = np.empty((DP, H), np.float32)
    for i in range(DP):
        Wr[:, i, :] = SCALE * (f("ln2_g")[i][:, None] * f("rconv_w")[i])
        br[i] = SCALE * (f("ln2_b")[i] @ f("rconv_w")[i] + f("rconv_b")[i])
    Wr2 = Wr.reshape(128, DP * H)
    # mean-centering of the layernorm folds into the weight itself
    Wr2 = Wr2 - np.ones((128, 1), np.float32) * (Wr2.sum(0, keepdims=True) / 128.0)
    Wqa = np.concatenate([Wc @ Wr2, (Wr2.T @ bc).reshape(1, 32)], axis=0)  # (27,32)
    w["W1"] = _bf(np.concatenate([Rc.T.astype(np.float32), Wqa], axis=1))  # (27,59)

    # joint encoder collapse 96->128
    Wj = f("je_w1") @ f("je_w2") @ f("je_w3")
    bj = (f("je_b1") @ f("je_w2") + f("je_b2")) @ f("je_w3") + f("je_b3")
    w["Wj"] = _bf(Wj)
    w["bj"] = np.ascontiguousarray(bj.reshape(128, 1))

    # per-depth block weights: ln1 folds into qkv (+ SCALE on q), ln3 into mw1
    qkvw = np.empty((DP, C, 3 * C), np.float32)
    qkvb = np.empty((DP, 3 * C), np.float32)
    mw1 = np.empty((DP, C, C), np.float32)
    mb1 = np.empty((DP, C), np.float32)
    for i in range(DP):
        qkvw[i] = f("ln1_g")[i][:, None] * f("qkv_w")[i]
        qkvb[i] = f("ln1_b")[i] @ f("qkv_w")[i] + f("qkv_b")[i]
        qkvw[i][:, :C] *= SCALE
        qkvb[i][:C] *= SCALE
        mw1[i] = f("ln3_g")[i][:, None] * f("mw1")[i]
        mb1[i] = f("ln3_b")[i] @ f("mw1")[i] + f("mb1")[i]
    # q/k head channels zero-padded to 32-aligned bases: head h -> group
    # g=h//3, slot j=h%3 at rows 32j..32j+16 of group tile; row 32j+16 is the
    # pad row used for the relation bias: q-pad=1, k-pad=br.
    qkw = np.zeros((C, DP, 2, 3, C), np.float32)
    qkbias = np.zeros((1, DP, 2, 3, C), np.float32)
    for i in range(DP):
        for t in range(2):
            wt = qkvw[i][:, t * C : (t + 1) * C]
            bt = qkvb[i][t * C : (t + 1) * C]
            for h in range(H):
                g, j = divmod(h, 3)
                qkw[:, i, t, g, 32 * j : 32 * j + HS] = wt[:, h * HS : (h + 1) * HS]
                qkbias[0, i, t, g, 32 * j : 32 * j + HS] = bt[h * HS : (h + 1) * HS]
                qkbias[0, i, t, g, 32 * j + HS] = 1.0 if t == 0 else br[i, h]
    w["qkw"] = _bf(qkw)
    w["qkbias"] = _bf(qkbias)
    w["vw"] = _bf(qkvw.transpose(1, 0, 2)[:, :, 2 * C :])  # (C, DP, C)
    w["vb_row"] = _bf(qkvb[:, 2 * C :].reshape(1, DP, C))
    projp = np.zeros((C, DP, 2, C), np.float32)
    for i in range(DP):
        for h in range(H):
            qq, m = divmod(h, 4)
            projp[32 * m : 32 * m + HS, i, qq, :] = f("proj_w")[i][
                h * HS : (h + 1) * HS, :
            ]
    w["projw"] = _bf(projp)  # (C, DP, 2, C), rows 32m+d of quad qq
    w["projb"] = np.ascontiguousarray(f("proj_b").T)  # (C, DP)
    w["mw1"] = _bf(mw1.transpose(1, 0, 2))
    w["mb1"] = np.ascontiguousarray(mb1.T)
    w["mw2"] = _bf(f("mw2").transpose(1, 0, 2))
    w["mb2"] = np.ascontiguousarray(f("mb2").T)

    # decoder collapse with final LN affine folded in
    Wdc = f("dw1") @ f("dw2") @ f("dw3")
    Wd = f("ng")[:, None] * Wdc
    bd = f("nb") @ Wdc + (f("db1") @ f("dw2") + f("db2")) @ f("dw3") + f("db3")
    w["Wd"] = _bf(Wd)
    w["bd"] = np.ascontiguousarray(bd.reshape(90, 1))

    # constants
    w["idr16"] = _bf(np.eye(128, dtype=np.float32))
    w["id32"] = np.eye(128, dtype=np.float32)
    w["o16"] = _bf(np.ones((1, 16), np.float32))
    # denominator-broadcast selectors: sel4[m, p2, c]: row m of pair-block
    # p2 -> cols 32*(2*p2+m)..+16
    sel = np.zeros((2, 2, 128), np.float32)
    for p2 in range(2):
        for m in range(2):
            sel[m, p2, 32 * (2 * p2 + m) : 32 * (2 * p2 + m) + 16] = 1.0
    w["sel4"] = _bf(sel)
    return w


NSUP = 32          # phase-1 tiles (2048 relation rows each)
G4 = 2             # groups of 1024 rows per tile
DP_EMIT = DP       # debug knob


def _build():
    nc = bacc.Bacc(None, target_bir_lowering=False)
    dram = {}

    def din(name, shape, dt=BF16):
        dram[name] = nc.dram_tensor(name, list(shape), dt, kind="ExternalInput")
        return dram[name]

    relT = din("relT", (NSUP, 27, 2048))
    connT = din("connT", (128, 2, N))
    jT = din("jT", (96, N))
    W1 = din("W1", (27, 59))
    Wj = din("Wj", (96, 128))
    bj = din("bj", (128, 1), F32)
    qkw = din("qkw", (C, DP, 2, 3, C))
    qkbias = din("qkbias", (1, DP, 2, 3, C))
    vw = din("vw", (C, DP, C))
    vb_row = din("vb_row", (1, DP, C))
    projw = din("projw", (C, DP, 2, C))
    projb = din("projb", (C, DP), F32)
    mw1 = din("mw1", (C, DP, C))
    mb1 = din("mb1", (C, DP), F32)
    mw2 = din("mw2", (C, DP, C))
    mb2 = din("mb2", (C, DP), F32)
    Wd = din("Wd", (128, 90))
    bd = din("bd", (90, 1), F32)
    idr16 = din("idr16", (128, 128))
    id32 = din("id32", (128, 128), F32)
    o16 = din("o16", (1, 16))
    sel4 = din("sel4", (2, 2, 128))
    out_d = nc.dram_tensor("out", [N, 90], F32, kind="ExternalOutput")

    from contextlib import ExitStack

    with tile.TileContext(nc) as tc, ExitStack() as ctx, nc.allow_low_precision(
        reason="bf16 matmul pipeline; end-to-end precision checked in test"
    ):
        const = ctx.enter_context(tc.tile_pool(name="const", bufs=1))

        # Pre-load the one activation table (natural_log_exp_and_others)
        # covering Ln/Exp/Square/Identity/Copy — the automatic inserter
        # would otherwise thrash between the natural_log and exp tables
        # on every LN rsqrt (1283ns per load).
        nc.scalar.add_instruction(
            mybir.InstLoadActFuncSet(
                name=nc.get_next_instruction_name(),
                ins=[],
                outs=[],
                act_func_set_id=6,
            )
        )

        def cload(h, shape, tag, dt=BF16):
            t = const.tile(list(shape), dt, tag=tag)
            nc.sync.dma_start(out=t, in_=h[:])
            return t

        W1_s = cload(W1, (27, 59), "W1")
        jT_s = cload(jT, (96, N), "jT")
        Wj_s = cload(Wj, (96, 128), "Wj")
        bj_s = cload(bj, (128, 1), "bj", F32)
        id32_s = cload(id32, (128, 128), "id32", F32)
        ones_s = const.tile([1, N], BF16, tag="ones_s")
        nc.vector.memset(ones_s[:], 1.0)
        eps_s = const.tile([128, 1], F32, tag="eps")
        nc.vector.memset(eps_s[:], 1e-5)
        # R[k, q, kt, ch]: relation logits (a * rsqrt(var)) in bf16
        R2 = const.tile([128, N, 2, 32], BF16, tag="R2")
        oTn = [
            const.tile([128, N], BF16, tag=f"oTn{qq}", name=f"oTn{qq}")
            for qq in range(2)
        ]
        for qq in range(2):
            nc.vector.memset(oTn[qq][:], 0.0)
        jf = [
            const.tile([128, 128], F32, tag=f"jf{qt}", name=f"jf{qt}")
            for qt in range(2)
        ]

        # ---------------- joint encoder -> jf (token-major) ---------------
        with tc.tile_pool(name="jenc", bufs=1, space="PSUM") as jpool:
            jp = jpool.tile([128, N], F32, tag="jp")
            nc.tensor.matmul(jp, Wj_s[:], jT_s[:], start=True, stop=True)
            jfT = const.tile([128, N], F32, tag="jfT")
            nc.scalar.activation(jfT, jp, AF.Identity, bias=bj_s[:])
            for qt in range(2):
                tp = jpool.tile([128, 128], F32, tag="jtp")
                nc.tensor.transpose(tp, jfT[:, qt * 128 : (qt + 1) * 128], id32_s[:])
                nc.vector.tensor_copy(out=jf[qt][:], in_=tp[:])

        # ---------------- Phase 1: relation branch -> R2 -------------------
        with tc.tile_pool(name="p1ps", bufs=2, space="PSUM") as pgp, \
             tc.tile_pool(name="p1sb", bufs=6) as p1sb, \
             tc.tile_pool(name="p1st", bufs=4) as p1st:
            pend = []
            mul_q = []  # deferred multiplies: (aa_tile, rsS, a, q0)
            sq = None
            nmul = 0

            def emit_muls():
                nonlocal nmul
                for aa_t, rs_t, a, q0 in mul_q:
                    eng = nc.vector
                    nmul += 1
                    eng.tensor_tensor(
                        out=R2[:, q0 : q0 + 8, :, :].rearrange(
                            "k (g q) kt c -> k g (q kt) c", g=G4
                        ),
                        in0=aa_t[:, :, :, :],
                        in1=rs_t[:, a, :, :, None].to_broadcast((128, G4, 8, 32)),
                        op=ALU.mult,
                    )
                mul_q.clear()

            for s in range(NSUP):
                relq = p1sb.tile([27, 2048], BF16, tag="relq")
                nc.sync.dma_start(out=relq, in_=relT[s])
                yc = pgp.tile([128, G4, 8, 32], F32, tag="yc", bufs=3)
                aa = pgp.tile([128, G4, 8, 32], F32, tag="aa", bufs=5)
                for g4 in range(G4):
                    for c in range(8):
                        k = g4 * 8 + c
                        xsl = relq[:, k * 128 : (k + 1) * 128]
                        nc.tensor.matmul(
                            yc[:, g4, c, 0:27], xsl, W1_s[:, 0:27],
                            start=True, stop=True,
                        )
                        nc.tensor.matmul(
                            aa[:, g4, c, :], xsl, W1_s[:, 27:59],
                            start=True, stop=True,
                        )
                if s % 2 == 0:
                    sq = p1sb.tile([128, 2, G4, 8, 27], BF16, tag="sq", bufs=3)
                nc.scalar.activation(sq[:, s % 2], yc[:, :, :, 0:27], AF.Square)
                pend.append(aa)
                if s % 2 == 1:
                    vS = p1st.tile([128, 2, G4, 8], F32, tag="vS")
                    nc.vector.tensor_reduce(
                        out=vS[:],
                        in_=sq[:],
                        axis=mybir.AxisListType.X,
                        op=ALU.add,
                    )
                    lnv = p1st.tile([128, 2, G4, 8], F32, tag="lnv")
                    nc.scalar.activation(
                        lnv.rearrange("p a g c -> p (a g c)"),
                        vS.rearrange("p a g c -> p (a g c)"),
                        AF.Ln,
                        bias=eps_s[:],
                    )
                    rsS = p1st.tile([128, 2, G4, 8], F32, tag="rsS")
                    nc.scalar.activation(
                        rsS.rearrange("p a g c -> p (a g c)"),
                        lnv.rearrange("p a g c -> p (a g c)"),
                        AF.Exp,
                        scale=-0.5,
                    )
                    nxt = [
                        (pga, rsS, a, (s - 1 + a) * 8)
                        for a, pga in enumerate(pend)
                    ]
                    emit_muls()
                    mul_q.extend(nxt)
                    pend = []
            emit_muls()


        # phase-2 weights: loaded behind the relation-tile DMAs so the first
        # relq arrives ~12us earlier
        qkw_s = cload(qkw, (C, DP, 2, 3, C), "qkw")
        qkb_s = cload(qkbias, (1, DP, 2, 3, C), "qkbias")
        vw_s = cload(vw, (C, DP, C), "vw")
        vbr_s = cload(vb_row, (1, DP, C), "vb_row")
        projw_s = cload(projw, (C, DP, 2, C), "projw")
        projb_s = cload(projb, (C, DP), "projb", F32)
        mw1_s = cload(mw1, (C, DP, C), "mw1")
        mb1_s = cload(mb1, (C, DP), "mb1", F32)
        mw2_s = cload(mw2, (C, DP, C), "mw2")
        mb2_s = cload(mb2, (C, DP), "mb2", F32)
        Wd_s = cload(Wd, (128, 90), "Wd")
        bd_s = cload(bd, (90, 1), "bd", F32)
        idr_s = cload(idr16, (128, 128), "idr16")
        o16_s = cload(o16, (1, 16), "o16")
        sel_s = cload(sel4, (2, 2, 128), "sel4")
        conn_s = cload(connT, (128, 2, N), "connT")

        # ---------------- Phase 2: 4 transformer blocks --------------------
        p2ps = ctx.enter_context(tc.tile_pool(name="p2ps", bufs=1, space="PSUM"))
        spps = ctx.enter_context(tc.tile_pool(name="spps", bufs=3, space="PSUM"))
        oqps = ctx.enter_context(tc.tile_pool(name="oqps", bufs=1, space="PSUM"))
        trps = ctx.enter_context(tc.tile_pool(name="trps", bufs=2, space="PSUM"))
        wrk = ctx.enter_context(tc.tile_pool(name="wrk", bufs=2))
        ekp = ctx.enter_context(tc.tile_pool(name="ekp", bufs=3))
        st = ctx.enter_context(tc.tile_pool(name="st", bufs=4))

        def layer_norm_t(tag):
            """Token-major standardize; returns feature-major (128,256) bf16.
            Both token halves advance stage-by-stage so no engine stream
            head-blocks on the other half."""
            xT = wrk.tile([128, N], BF16, tag=f"xT_{tag}")
            st6, mv, lnv1, rs1, xh, tp = [[None, None] for _ in range(6)]
            for qt in range(2):
                st6[qt] = st.tile([128, 6], F32, tag="st6", name="st6")
                nc.vector.bn_stats(out=st6[qt], in_=jf[qt][:])
            for qt in range(2):
                mv[qt] = st.tile([128, 2], F32, tag="mv", name="mv")
                nc.vector.bn_aggr(out=mv[qt], in_=st6[qt][:])
            for qt in range(2):
                lnv1[qt] = st.tile([128, 1], F32, tag="lnv1", name="lnv1")
                nc.scalar.activation(
                    lnv1[qt], mv[qt][:, 1:2], AF.Ln, bias=eps_s[:]
                )
                rs1[qt] = st.tile([128, 1], F32, tag="rs1", name="rs1")
                nc.scalar.activation(rs1[qt], lnv1[qt][:], AF.Exp, scale=-0.5)
            for qt in range(2):
                xh[qt] = st.tile([128, 128], BF16, tag="xh", name="xh")
                nc.vector.tensor_scalar(
                    out=xh[qt], in0=jf[qt][:], scalar1=mv[qt][:, 0:1],
                    scalar2=rs1[qt][:], op0=ALU.subtract, op1=ALU.mult,
                )
            for qt in range(2):
                tp[qt] = trps.tile([128, 128], BF16, tag="sm", name="lntp")
                nc.tensor.transpose(tp[qt], xh[qt][:], idr_s[:])
                if qt == 0:
                    nc.scalar.activation(
                        xT[:, qt * 128 : (qt + 1) * 128], tp[qt], AF.Copy
                    )
                else:
                    nc.vector.tensor_copy(
                        out=xT[:, qt * 128 : (qt + 1) * 128], in_=tp[qt][:]
                    )
            return xT

        for i in range(DP_EMIT):
            xT = layer_norm_t(f"ln1_{i}")
            # qkv
            qkT = []
            for t in range(2):
                qk_ps = p2ps.tile([128, 3, N], F32, tag="qk_ps")
                for g in range(3):
                    for half in range(2):
                        hs = slice(half * 128, (half + 1) * 128)
                        nc.tensor.matmul(
                            qk_ps[:, g, hs], qkb_s[:, i, t, g, :],
                            ones_s[:, hs], start=True, stop=False,
                        )
                        nc.tensor.matmul(
                            qk_ps[:, g, hs], qkw_s[:, i, t, g, :], xT[:, hs],
                            start=False, stop=True,
                        )
                qkt = wrk.tile([128, 3, N], BF16, tag=f"qkt{t}")
                nc.scalar.activation(qkt, qk_ps, AF.Copy)
                qkT.append(qkt)
            # v directly token-major: out (tok, ch) per kt half
            vx = wrk.tile([128, 2, H, 16], BF16, tag="vx")
            ones_c = wrk.tile([128, 1], BF16, tag="ones_c")
            nc.vector.memset(ones_c[:], 1.0)
            for kt in range(2):
                v_ps = trps.tile([128, 128], F32, tag="sm", name="v_ps")
                nc.tensor.matmul(
                    v_ps, ones_s[:, kt * 128 : kt * 128 + 128], vbr_s[:, i, :],
                    start=True, stop=False,
                )
                nc.tensor.matmul(
                    v_ps, xT[:, kt * 128 : (kt + 1) * 128], vw_s[:, i, :],
                    start=False, stop=True,
                )
                nc.vector.tensor_copy(
                    out=vx[:, kt, :, :],
                    in_=v_ps.rearrange("p (h d) -> p h d", h=H),
                )

            oq_tiles = {}
            dn_tiles = {}

            def eval_head(h):
                ih = i * 8 + h
                g, j = divmod(h, 3)
                hp = slice(32 * j, 32 * j + HS + 1)
                sp = spps.tile([128, 2, N], F32, tag="sp")
                for kt in range(2):
                    nc.tensor.matmul(
                        sp[:, kt, :],
                        qkT[1][hp, g, kt * 128 : (kt + 1) * 128],
                        qkT[0][hp, g, :],
                        start=True, stop=False,
                    )
                    nc.tensor.matmul(
                        sp[:, kt, :], idr_s[:], R2[:, :, kt, ih],
                        start=False, stop=True,
                    )
                nc.vector.tensor_tensor(
                    out=sp[:], in0=sp[:], in1=conn_s[:], op=ALU.mult
                )
                ek = ekp.tile([128, 2, N], BF16, tag="ek")
                nc.scalar.activation(ek, sp, AF.Exp)
                q4, m = divmod(h, 4)
                if m == 0:
                    oq_tiles[q4] = oqps.tile([128, N], F32, tag="oq", name="oq")
                    nc.vector.memset(oq_tiles[q4][:], 0.0)
                    dn_tiles[q4] = trps.tile(
                        [128, N], F32, tag="sm", name="dn_ps"
                    )
                oq = oq_tiles[q4]
                dn = dn_tiles[q4]
                for kt in range(2):
                    nc.tensor.matmul(
                        oq[32 * m : 32 * m + HS, :],
                        vx[:, kt, h, :],
                        ek[:, kt, :],
                        start=(kt == 0), stop=(kt == 1),
                        tile_position=(0, 32 * m),
                    )
                    nc.tensor.matmul(
                        dn[32 * m : 32 * m + 1, :],
                        ones_c[:],
                        ek[:, kt, :],
                        start=(kt == 0), stop=(kt == 1),
                        tile_position=(0, 32 * m),
                    )

            def norm_quad2(q4):
                oq = oq_tiles[q4]
                dn = dn_tiles[q4]
                oqsb = st.tile([128, N], BF16, tag="oqsb", name="oqsb")
                nc.scalar.activation(oqsb, oq, AF.Copy)
                dvv = [None] * 4
                for m in range(4):
                    dvv[m] = st.tile([1, N], BF16, tag="dvv", name="dvv")
                    nc.vector.reciprocal(
                        out=dvv[m][:], in_=dn[32 * m : 32 * m + 1, :]
                    )
                dh_ps = trps.tile([128, N], F32, tag="sm", name="dh_ps")
                for m in range(4):
                    nc.tensor.matmul(
                        dh_ps[32 * m : 32 * m + 16, :], o16_s[:], dvv[m][:],
                        start=True, stop=True, tile_position=(0, 32 * m),
                    )
                for m in range(4):
                    nc.vector.tensor_tensor(
                        out=oTn[q4][32 * m : 32 * m + 16, :],
                        in0=oqsb[32 * m : 32 * m + 16, :],
                        in1=dh_ps[32 * m : 32 * m + 16, :],
                        op=ALU.mult,
                    )

            prb = p2ps.tile([128, 3, N], F32, tag="qk_ps", name="prb")
            pr_ps = prb[:, 0, :]
            for h in range(H):
                eval_head(h)
                if h == 3:
                    norm_quad2(0)
                if h == 7:
                    norm_quad2(1)
                if h == 5:
                    nc.tensor.matmul(
                        pr_ps, projw_s[:, i, 0, :], oTn[0][:],
                        start=True, stop=False,
                    )
            nc.tensor.matmul(
                pr_ps, projw_s[:, i, 1, :], oTn[1][:],
                start=False, stop=True,
            )
            prT = wrk.tile([128, N], BF16, tag="prT")
            rtp = [None, None]
            for qt in range(2):
                nc.scalar.activation(
                    prT[:, qt * 128 : (qt + 1) * 128],
                    pr_ps[:, qt * 128 : (qt + 1) * 128],
                    AF.Identity, bias=projb_s[:, i : i + 1],
                )
            for qt in range(2):
                rtp[qt] = trps.tile([128, 128], BF16, tag="sm", name="rtp")
                nc.tensor.transpose(
                    rtp[qt], prT[:, qt * 128 : (qt + 1) * 128], idr_s[:]
                )
            for qt in range(2):
                nc.vector.tensor_tensor(
                    out=jf[qt][:], in0=jf[qt][:], in1=rtp[qt][:], op=ALU.add
                )

            # MLP
            xT3 = layer_norm_t(f"ln3_{i}")
            h1b = p2ps.tile([128, 3, N], F32, tag="qk_ps", name="h1b")
            h1_ps = h1b[:, 0, :]
            h2b = p2ps.tile([128, 3, N], F32, tag="qk_ps", name="h2b")
            h2_ps = h2b[:, 0, :]
            h1sb = wrk.tile([128, N], F32, tag="h1sb")
            e16 = wrk.tile([128, N], F32, tag="e16")
            ep = wrk.tile([128, N], F32, tag="ep")
            r32 = wrk.tile([128, N], F32, tag="r32")
            g16 = wrk.tile([128, N], BF16, tag="g16")
            h2T = wrk.tile([128, N], BF16, tag="h2T")
            # gelu(x) ~= x * sigmoid(1.5957691 x), two per-token-half chains
            # advanced stage-by-stage (no engine stream head-blocking)
            halves = [slice(0, 128), slice(128, 256)]
            for qt, hs in enumerate(halves):
                nc.tensor.matmul(
                    h1_ps[:, hs], mw1_s[:, i, :], xT3[:, hs],
                    start=True, stop=True,
                )
            for qt, hs in enumerate(halves):
                nc.scalar.activation(
                    h1sb[:, hs], h1_ps[:, hs], AF.Identity,
                    bias=mb1_s[:, i : i + 1],
                )
                nc.scalar.activation(
                    e16[:, hs], h1_ps[:, hs], AF.Exp, scale=-1.5957691
                )
            for qt, hs in enumerate(halves):
                nc.vector.tensor_scalar(
                    out=ep[:, hs], in0=e16[:, hs], scalar1=1.0, scalar2=None,
                    op0=ALU.add,
                )
            for qt, hs in enumerate(halves):
                nc.vector.reciprocal(out=r32[:, hs], in_=ep[:, hs])
            for qt, hs in enumerate(halves):
                nc.vector.tensor_tensor(
                    out=g16[:, hs], in0=h1sb[:, hs], in1=r32[:, hs],
                    op=ALU.mult,
                )
            for qt, hs in enumerate(halves):
                nc.tensor.matmul(
                    h2_ps[:, hs], mw2_s[:, i, :], g16[:, hs],
                    start=True, stop=True,
                )
            for qt, hs in enumerate(halves):
                nc.scalar.activation(
                    h2T[:, hs], h2_ps[:, hs], AF.Identity,
                    bias=mb2_s[:, i : i + 1],
                )
            mtp = [None, None]
            for qt, hs in enumerate(halves):
                mtp[qt] = trps.tile([128, 128], BF16, tag="sm", name="rtp")
                nc.tensor.transpose(mtp[qt], h2T[:, hs], idr_s[:])
            for qt in range(2):
                nc.vector.tensor_tensor(
                    out=jf[qt][:], in0=jf[qt][:], in1=mtp[qt][:], op=ALU.add
                )

        # ---------------- decoder ------------------------------------------
        xTf = layer_norm_t("lnf")
        opb = p2ps.tile([128, 3, N], F32, tag="qk_ps", name="opb")
        op_ps = opb[0:90, 0, :]
        nc.tensor.matmul(op_ps, Wd_s[:], xTf[:], start=True, stop=True)
        outT = wrk.tile([90, N], F32, tag="outT")
        nc.scalar.activation(outT, op_ps, AF.Identity, bias=bd_s[:])
        for qt in range(2):
            tp = trps.tile([128, 90], F32, tag="sm", name="otp")
            nc.tensor.transpose(
                tp, outT[:, qt * 128 : (qt + 1) * 128], id32_s[:90, :90]
            )
            of = wrk.tile([128, 90], F32, tag="of")
            nc.vector.tensor_copy(out=of[:], in_=tp[:])
            nc.sync.dma_start(out=out_d[qt * 128 : (qt + 1) * 128, :], in_=of[:])

    nc.compile()
    return nc


def kernel(**inputs):
    global last_results
    w = _fold(inputs)
    rel = np.asarray(inputs["relation_in"], np.float32)
    conn = np.asarray(inputs["conn"], np.float32)
    joint = np.asarray(inputs["joint_in"], np.float32)

    in_maps = []
    for b in range(B):
        m = dict(w)
        rT = np.empty((27, NN), np.float32)
        rT[0:26] = rel[b].reshape(NN, 26).T
        rT[26] = 1.0
        m["relT"] = np.ascontiguousarray(
            rT.reshape(27, NSUP, 2048).transpose(1, 0, 2)
        ).astype(_BF)
        # connT[k%128, k//128, q] = conn[b][q, k]
        cT = conn[b].T.reshape(2, 128, N).transpose(1, 0, 2)
        m["connT"] = _bf(cT)
        m["jT"] = _bf(joint[b].T)
        in_maps.append(m)

    nc = _build()
    last_results = run_bass_kernel_spmd(nc, in_maps, core_ids=list(range(B)))
    out = np.stack([r["out"] for r in last_results.results])
    return out.astype(np.float32)
